# revision 1
# baseline (speedup 1.0000x reference)
"""AtIndexPooler (embedding lookup) on 8 TRN2 NeuronCores.

Data-parallel along batch: each core owns B/8 = 64 batch rows. Per core the
hidden_state shard is viewed as a flat row table [64*512, 1024] with the two
missing-embedding rows appended at the end ([32770, 1024] total). The host
turns indices into flat row offsets (invalid index -1 -> appended missing
row); the device performs the lookup as one full-width 128-row indirect DMA
gather (one 4KB row per SBUF partition) followed by two parallel stores of
the pooled output on the two HWDGE rings.

Hardware notes baked into this design (all verified on TRN2 silicon):
- A partial-partition indirect DMA only has descriptors on the SDMA engines
  wired to those partitions' SBUF ports (64 contiguous partitions -> 8 of 16
  engines -> semaphore only reaches 8), and two back-to-back partial
  indirects leave the device unrecoverable. Every indirect here spans all
  128 partitions.
- The indirect offset table must be [128, 1] int32, one offset per
  partition; [1,128]/[64,2]/[32,4] layouts fail or corrupt on HW.
- Splitting the gather along the hidden dim (half-rows) doubles the serial
  Q7 descriptor generation (~11ns/descriptor), which costs more than the
  gather/store overlap it enables.
"""

import sys

import numpy as np

if "/opt/trn_rl_repo" not in sys.path:
    sys.path.insert(0, "/opt/trn_rl_repo")

from concourse import bacc, bass, mybir
from concourse.bass_utils import run_bass_kernel_spmd

BATCH, SEQ_LEN, HIDDEN = 512, 512, 1024
NUM_INDICES = 2
N_CORES = 8
B_SHARD = BATCH // N_CORES                # 64 batches per core
ROWS = B_SHARD * NUM_INDICES              # 128 gather rows = 128 partitions
DATA_ROWS = B_SHARD * SEQ_LEN + NUM_INDICES  # 32770 rows in the lookup table

_NC_CACHE = None
LAST_RESULT = None  # BassKernelResults of the most recent run (for profiling)


def _build_nc():
    HALF = ROWS // 2
    nc = bacc.Bacc("TRN2", target_bir_lowering=False, debug=False, num_devices=N_CORES)
    data = nc.dram_tensor("data", [DATA_ROWS, HIDDEN], mybir.dt.float32, kind="ExternalInput")
    offs = nc.dram_tensor("offs", [ROWS, 1], mybir.dt.int32, kind="ExternalInput")
    out = nc.dram_tensor("out", [ROWS, HIDDEN], mybir.dt.float32, kind="ExternalOutput")

    sA = nc.alloc_semaphore("sA")    # offs load completion
    sB = nc.alloc_semaphore("sB")    # gather completion
    sC0 = nc.alloc_semaphore("sC0")  # store half 0 completion
    sC1 = nc.alloc_semaphore("sC1")  # store half 1 completion
    offs_sb = nc.alloc_sbuf_tensor("offs_sb", [ROWS, 1], mybir.dt.int32)
    gath = nc.alloc_sbuf_tensor("gath", [ROWS, HIDDEN], mybir.dt.float32)

    nc.sync.dma_start(out=offs_sb[:, :], in_=offs[:, :], single_packet=True).then_inc(sA, 16)

    nc.gpsimd.wait_ge(sA, 16)
    nc.gpsimd.indirect_dma_start(
        out=gath[:, :],
        out_offset=None,
        in_=data[:, :],
        in_offset=bass.IndirectOffsetOnAxis(ap=offs_sb[:, :1], axis=0),
    ).then_inc(sB, 16)

    # halves drain in parallel: rows 0-63 read via the even SDMA engines on
    # the SP ring, rows 64-127 via the odd engines on the ACT ring
    nc.sync.wait_ge(sB, 16)
    nc.sync.dma_start(out=out[:HALF, :], in_=gath[:HALF, :]).then_inc(sC0, 16)
    nc.scalar.wait_ge(sB, 16)
    nc.scalar.dma_start(out=out[HALF:, :], in_=gath[HALF:, :]).then_inc(sC1, 16)

    for s in (sA, sB, sC0, sC1):
        nc.sync.wait_ge(s, 16)
    nums = sorted(s.num for s in (sA, sB, sC0, sC1))
    assert nums == list(range(nums[0], nums[0] + 4))
    nc.sync.sem_clear(range(nums[0], nums[-1] + 1))

    nc.compile()
    return nc


def kernel(hidden_state, missing_embeddings, indices):
    global _NC_CACHE, LAST_RESULT
    hidden_state = np.ascontiguousarray(np.asarray(hidden_state, dtype=np.float32))
    missing_embeddings = np.ascontiguousarray(np.asarray(missing_embeddings, dtype=np.float32))
    indices = np.asarray(indices)

    if _NC_CACHE is None:
        _NC_CACHE = _build_nc()
    nc = _NC_CACHE

    base = (np.arange(B_SHARD, dtype=np.int64) * SEQ_LEN)[:, None]
    miss_rows = B_SHARD * SEQ_LEN + np.arange(NUM_INDICES, dtype=np.int64)[None, :]
    in_maps = []
    for c in range(N_CORES):
        hs = hidden_state[c * B_SHARD : (c + 1) * B_SHARD].reshape(B_SHARD * SEQ_LEN, HIDDEN)
        idx = indices[c * B_SHARD : (c + 1) * B_SHARD].astype(np.int64)  # [64, 2]
        flat = np.where(idx >= 0, base + np.clip(idx, 0, SEQ_LEN - 1), miss_rows).reshape(ROWS)
        data = np.concatenate([hs, missing_embeddings], axis=0)
        offs = flat.astype(np.int32).reshape(ROWS, 1)
        in_maps.append({"data": data, "offs": offs})

    LAST_RESULT = run_bass_kernel_spmd(nc, in_maps, core_ids=list(range(N_CORES)))
    outs = [
        LAST_RESULT.results[c]["out"].reshape(B_SHARD, NUM_INDICES * HIDDEN)
        for c in range(N_CORES)
    ]
    return np.concatenate(outs, axis=0)



# revision 2
# speedup vs baseline: 1.3030x; 1.3030x over previous
"""AtIndexPooler (embedding lookup) on 8 TRN2 NeuronCores.

Data-parallel along batch: each core owns B/8 = 64 batch rows. Per core the
hidden_state shard is viewed as a flat row table [64*512, 1024] with the two
missing-embedding rows appended at the end ([32770, 1024] total). The host
turns indices into flat row offsets (invalid index -1 -> appended missing
row); the device performs the lookup as one full-width 128-row indirect DMA
gather (one 4KB row per SBUF partition) followed by a single 128-partition
store of the pooled output.

Hardware notes baked into this design (all verified on TRN2 silicon):
- A partial-partition indirect DMA only has descriptors on the SDMA engines
  wired to those partitions' SBUF ports (64 contiguous partitions -> 8 of 16
  engines -> semaphore only reaches 8), and two back-to-back partial
  indirects leave the device unrecoverable. Every indirect here spans all
  128 partitions.
- The indirect offset table must be [128, 1] int32, one offset per
  partition; [1,128]/[64,2]/[32,4] layouts fail or corrupt on HW.
- ~7.6us of the measured kernel span is NEFF/NRT prologue+epilogue
  (instruction-fetch wait, two engine sem-sync rounds, const memsets +
  all-engine barrier, final semaphore zeroing). The memsets + barrier are
  emitted by Bass.__init__ before user code; they are deleted from the IR
  below so the offs load issues right after the fixed rust preamble and
  overlaps what remains of the prologue.
- enable_partition_id=False drops a ~1us TENSOR_LOAD from the prologue.
"""

import sys

import numpy as np

if "/opt/trn_rl_repo" not in sys.path:
    sys.path.insert(0, "/opt/trn_rl_repo")

from concourse import bacc, bass, mybir
from concourse.bass_utils import run_bass_kernel_spmd

BATCH, SEQ_LEN, HIDDEN = 512, 512, 1024
NUM_INDICES = 2
N_CORES = 8
B_SHARD = BATCH // N_CORES                # 64 batches per core
ROWS = B_SHARD * NUM_INDICES              # 128 gather rows = 128 partitions
DATA_ROWS = B_SHARD * SEQ_LEN + NUM_INDICES  # 32770 rows in the lookup table

_NC_CACHE = None
LAST_RESULT = None  # BassKernelResults of the most recent run (for profiling)


def _strip_init_preamble(nc):
    """Remove the const-AP memsets and the init all-engine barrier emitted by
    Bass.__init__. Nothing in this kernel reads the const tensors, every DMA
    is semaphore-gated, and NRT serializes executions, so the barrier only
    delays the first user instruction (~1.5us on the measured critical path).
    """
    blk = nc.main_func.blocks[0]
    drop = []
    for i in blk.instructions:
        if isinstance(i, mybir.InstMemset):
            drop.append(i)
        elif isinstance(i, mybir.InstDrain):
            drop.append(i)
        elif isinstance(i, mybir.InstEventSemaphore) and i.name.startswith("barrier_"):
            drop.append(i)
    for i in drop:
        blk.instructions.remove(i)
        nc.inst_map.pop(i.name, None)


def _build_nc():
    nc = bacc.Bacc(
        "TRN2",
        target_bir_lowering=False,
        debug=False,
        num_devices=N_CORES,
        enable_partition_id=False,
        monotonic_sem_count=0,
    )
    data = nc.dram_tensor("data", [DATA_ROWS, HIDDEN], mybir.dt.float32, kind="ExternalInput")
    offs = nc.dram_tensor("offs", [ROWS, 1], mybir.dt.int32, kind="ExternalInput")
    out = nc.dram_tensor("out", [ROWS, HIDDEN], mybir.dt.float32, kind="ExternalOutput")

    sA = nc.alloc_semaphore("sA")    # offs load completion
    sB = nc.alloc_semaphore("sB")    # gather completion
    sC = nc.alloc_semaphore("sC")    # store completion
    offs_sb = nc.alloc_sbuf_tensor("offs_sb", [ROWS, 1], mybir.dt.int32)
    gath = nc.alloc_sbuf_tensor("gath", [ROWS, HIDDEN], mybir.dt.float32)

    _strip_init_preamble(nc)

    # Issued as SP's first post-preamble instruction; overlaps the remaining
    # NEFF prologue on the other engines.
    nc.sync.dma_start(out=offs_sb[:, :], in_=offs[:, :], single_packet=True).then_inc(sA, 16)

    nc.gpsimd.wait_ge(sA, 16)
    nc.gpsimd.indirect_dma_start(
        out=gath[:, :],
        out_offset=None,
        in_=data[:, :],
        in_offset=bass.IndirectOffsetOnAxis(ap=offs_sb[:, :1], axis=0),
    ).then_inc(sB, 16)

    nc.sync.wait_ge(sB, 16)
    nc.sync.dma_start(out=out[:, :], in_=gath[:, :]).then_inc(sC, 16)

    # sC>=16 implies the store issued, which implies sB==16, which implies
    # sA==16 (the gather waited on it) — one wait covers the chain.
    nc.sync.wait_ge(sC, 16)
    nums = sorted(s.num for s in (sA, sB, sC))
    assert nums == list(range(nums[0], nums[0] + 3))
    nc.sync.sem_clear(range(nums[0], nums[-1] + 1))

    nc.compile()
    return nc


def kernel(hidden_state, missing_embeddings, indices):
    global _NC_CACHE, LAST_RESULT
    hidden_state = np.ascontiguousarray(np.asarray(hidden_state, dtype=np.float32))
    missing_embeddings = np.ascontiguousarray(np.asarray(missing_embeddings, dtype=np.float32))
    indices = np.asarray(indices)

    if _NC_CACHE is None:
        _NC_CACHE = _build_nc()
    nc = _NC_CACHE

    base = (np.arange(B_SHARD, dtype=np.int64) * SEQ_LEN)[:, None]
    miss_rows = B_SHARD * SEQ_LEN + np.arange(NUM_INDICES, dtype=np.int64)[None, :]
    in_maps = []
    for c in range(N_CORES):
        hs = hidden_state[c * B_SHARD : (c + 1) * B_SHARD].reshape(B_SHARD * SEQ_LEN, HIDDEN)
        idx = indices[c * B_SHARD : (c + 1) * B_SHARD].astype(np.int64)  # [64, 2]
        flat = np.where(idx >= 0, base + np.clip(idx, 0, SEQ_LEN - 1), miss_rows).reshape(ROWS)
        data = np.concatenate([hs, missing_embeddings], axis=0)
        offs = flat.astype(np.int32).reshape(ROWS, 1)
        in_maps.append({"data": data, "offs": offs})

    LAST_RESULT = run_bass_kernel_spmd(nc, in_maps, core_ids=list(range(N_CORES)))
    outs = [
        LAST_RESULT.results[c]["out"].reshape(B_SHARD, NUM_INDICES * HIDDEN)
        for c in range(N_CORES)
    ]
    return np.concatenate(outs, axis=0)
